# revision 1
# baseline (speedup 1.0000x reference)
"""Graphormer3D encoder layer on 8 Trainium2 NeuronCores.

Sharding: data-parallel over the 16 graphs (2 per core); params replicated.
On-chip layout is feature-major (x^T: [feature, token]) in fp16 with fp32 PSUM
accumulation:
  - LayerNorm mean/var via ones-matmul partition reductions on TensorE
    (stats broadcast across partitions for free), rsqrt as exp(-0.5*ln(var)).
  - QKV/out/fc matmuls keep activations feature-major (lhsT = W^T chunks).
  - attn_bias added into the scores PSUM with an identity-matmul accumulate.
  - softmax: ACT exp with fused row-sum accum, DVE reciprocal + per-partition
    scale; probs/v transposed on TensorE for the probs@v contraction.
All matmuls use N<=512 so each accumulation group stays in one PSUM bank.
"""
import numpy as np

N_NODE, N_GRAPH, D = 512, 16, 768
H, HD, FFN = 8, 96, 3072
EPS = 1e-5
NC = 8            # cores
G = 2             # graphs per core
T = G * N_NODE    # tokens per core (1024)
KC = D // 128     # 6 feature chunks
FC = FFN // 128   # 24 ffn chunks
NQT = N_NODE // 128  # 4 q/k tiles per graph
HLF = (slice(0, 512), slice(512, 1024))

_cached = {}


def _build():
    import concourse.bass as bass
    import concourse.mybir as mybir
    import concourse.tile as tile
    import concourse.bacc as bacc
    from contextlib import ExitStack

    F16 = mybir.dt.float16
    F32 = mybir.dt.float32
    AF = mybir.ActivationFunctionType
    OP = mybir.AluOpType

    nc = bacc.Bacc("TRN2", target_bir_lowering=False, debug=False, num_devices=NC)

    di = lambda name, shape, dt: nc.declare_dram_parameter(name, shape, dt, isOutput=False)
    xt_d = di("xt", [KC, 128, T], F16)
    bias_d = di("biasb", [G * H, N_NODE, N_NODE], F16)
    mask_d = di("maskrow", [1, T], F16)
    wqkv_d = di("wqkv", [KC, 128, 3 * D], F16)
    bqkv_d = di("bqkv", [HD, 3 * H], F32)
    wout_d = di("wout", [H, HD, D], F16)
    bout_d = di("bout", [128, KC], F32)
    wfc1_d = di("wfc1", [KC, 128, FFN], F16)
    bfc1_d = di("bfc1", [128, FC], F32)
    wfc2_d = di("wfc2", [FC, 128, D], F16)
    bfc2_d = di("bfc2", [128, KC], F32)
    g1_d = di("g1", [128, KC], F32)
    b1_d = di("b1", [128, KC], F32)
    g2_d = di("g2", [128, KC], F32)
    b2_d = di("b2", [128, KC], F32)
    ident_d = di("ident", [128, 128], F16)
    ones_d = di("ones", [128, 128], F16)
    yt_d = nc.declare_dram_parameter("yt", [KC, 128, T], F32, isOutput=True)

    with tile.TileContext(nc) as tc, ExitStack() as top:
        const = top.enter_context(tc.tile_pool(name="const", bufs=1))

        def load_const(name, dram, shape, dt):
            t = const.tile(shape, dt, tag=name)
            nc.sync.dma_start(t[:], dram[:])
            return t

        ident = load_const("ident", ident_d, [128, 128], F16)
        ones = load_const("ones", ones_d, [128, 128], F16)
        mask_sb = load_const("mask", mask_d, [1, T], F16)
        bqkv = load_const("bqkv", bqkv_d, [HD, 3 * H], F32)
        bout = load_const("bout", bout_d, [128, KC], F32)
        bfc1 = load_const("bfc1", bfc1_d, [128, FC], F32)
        bfc2 = load_const("bfc2", bfc2_d, [128, KC], F32)
        g1 = load_const("g1", g1_d, [128, KC], F32)
        b1 = load_const("b1", b1_d, [128, KC], F32)
        g2 = load_const("g2", g2_d, [128, KC], F32)
        b2 = load_const("b2", b2_d, [128, KC], F32)
        eps_sb = const.tile([128, 1], F32, tag="eps")
        nc.vector.memset(eps_sb[:], EPS)

        # persistent activation pools (LIFO pool stack: opens/closes nest)
        h_pool = top.enter_context(tc.tile_pool(name="h", bufs=KC))
        y1_pool = top.enter_context(tc.tile_pool(name="y1", bufs=KC))
        stat_pool = top.enter_context(tc.tile_pool(name="stat", bufs=1))
        sq_pool = top.enter_context(tc.tile_pool(name="sq", bufs=2))
        tmp_pool = top.enter_context(tc.tile_pool(name="tmp", bufs=2))
        yo_pool = top.enter_context(tc.tile_pool(name="yo", bufs=2))
        s_x = ExitStack()
        s_qkv = ExitStack()
        s_attn = ExitStack()
        s_wout = ExitStack()
        x_pool = s_x.enter_context(tc.tile_pool(name="x", bufs=KC))

        def layer_norm(x_tiles, g_t, b_t, psum_pool, h_tag):
            """feature-major LN over the partition axis: returns KC fp16 tiles"""
            sq_tiles = []
            for k in range(KC):
                sq = sq_pool.tile([128, T], F16, tag="sq")
                nc.vector.tensor_tensor(sq[:], x_tiles[k][:], x_tiles[k][:], op=OP.mult)
                sq_tiles.append(sq)
            ps_s = psum_pool.tile([128, T], F32, tag="mm")
            for k in range(KC):
                for hf in range(2):
                    nc.tensor.matmul(ps_s[:, HLF[hf]], ones[:], x_tiles[k][:, HLF[hf]],
                                     start=(k == 0), stop=(k == KC - 1))
            ps_q = psum_pool.tile([128, T], F32, tag="mm")
            for k in range(KC):
                for hf in range(2):
                    nc.tensor.matmul(ps_q[:, HLF[hf]], ones[:], sq_tiles[k][:, HLF[hf]],
                                     start=(k == 0), stop=(k == KC - 1))
            mu = stat_pool.tile([128, T], F32, tag="mu")
            nc.vector.tensor_scalar_mul(mu[:], ps_s[:], 1.0 / D)
            ms = stat_pool.tile([128, T], F32, tag="ms")
            nc.vector.tensor_scalar_mul(ms[:], ps_q[:], 1.0 / D)
            var = stat_pool.tile([128, T], F32, tag="var")
            nc.vector.tensor_tensor(var[:], mu[:], mu[:], op=OP.mult)
            nc.vector.tensor_tensor(var[:], ms[:], var[:], op=OP.subtract)
            lnv = stat_pool.tile([128, T], F32, tag="lnv")
            nc.scalar.activation(lnv[:], var[:], AF.Ln, bias=eps_sb[:])
            rs = stat_pool.tile([128, T], F16, tag="rs")
            nc.scalar.activation(rs[:], lnv[:], AF.Exp, scale=-0.5)
            mu16 = stat_pool.tile([128, T], F16, tag="mu16")
            nc.vector.tensor_copy(mu16[:], mu[:])
            h_tiles = []
            for k in range(KC):
                ht = h_pool.tile([128, T], F16, tag=h_tag)
                nc.vector.tensor_tensor(ht[:], x_tiles[k][:], mu16[:], op=OP.subtract)
                nc.vector.tensor_tensor(ht[:], ht[:], rs[:], op=OP.mult)
                nc.vector.tensor_scalar(ht[:], ht[:], g_t[:, k:k + 1], b_t[:, k:k + 1],
                                        op0=OP.mult, op1=OP.add)
                h_tiles.append(ht)
            return h_tiles

        # ---------------- phase 1: load x, LN1, QKV ----------------
        x_tiles = []
        for k in range(KC):
            xt = x_pool.tile([128, T], F16, tag="x")
            nc.sync.dma_start(xt[:], xt_d[k])
            x_tiles.append(xt)

        attn_pool = s_attn.enter_context(tc.tile_pool(name="attn", bufs=1))
        attn_sb = attn_pool.tile([HD, H * T], F16, tag="attn")
        qkv_pool = s_qkv.enter_context(tc.tile_pool(name="qkv", bufs=1))
        q_sb = qkv_pool.tile([HD, H * T], F16, tag="q")
        k_sb = qkv_pool.tile([HD, H * T], F16, tag="k")
        v_sb = qkv_pool.tile([HD, H * T], F16, tag="v")
        qkv_sbs = [q_sb, k_sb, v_sb]

        with tc.tile_pool(name="wqkv", bufs=KC) as wqkv_pool, \
             tc.tile_pool(name="ps_a", bufs=2, space="PSUM") as ps_a:
            wq_tiles = []
            for k in range(KC):
                wt = wqkv_pool.tile([128, 3 * D], F16, tag="wqkv")
                nc.sync.dma_start(wt[:], wqkv_d[k])
                wq_tiles.append(wt)

            h1 = layer_norm(x_tiles, g1, b1, ps_a, "h")

            for tau in range(3):
                for hh in range(H):
                    th = tau * H + hh
                    ps = ps_a.tile([HD, T], F32, tag="qkv")
                    for hf in range(2):
                        for k in range(KC):
                            nc.tensor.matmul(
                                ps[:, HLF[hf]], wq_tiles[k][:, th * HD:(th + 1) * HD],
                                h1[k][:, HLF[hf]], start=(k == 0), stop=(k == KC - 1))
                    dst = qkv_sbs[tau][:, hh * T:(hh + 1) * T]
                    nc.scalar.activation(dst, ps[:], AF.Identity, bias=bqkv[:, th:th + 1])

        # ---------------- phase 2: attention ----------------
        with tc.tile_pool(name="biasbuf", bufs=6) as bias_pool, \
             tc.tile_pool(name="probs", bufs=9) as prob_pool, \
             tc.tile_pool(name="pt", bufs=6) as pt_pool, \
             tc.tile_pool(name="vt", bufs=2) as vt_pool, \
             tc.tile_pool(name="small", bufs=3) as small_pool, \
             tc.tile_pool(name="ps_sc", bufs=3, space="PSUM") as ps_sc, \
             tc.tile_pool(name="ps_pt", bufs=2, space="PSUM") as ps_pt, \
             tc.tile_pool(name="ps_vt", bufs=1, space="PSUM") as ps_vt, \
             tc.tile_pool(name="ps_at", bufs=2, space="PSUM") as ps_at:
            for g in range(G):
                for hh in range(H):
                    gh = g * H + hh
                    base = hh * T + g * N_NODE
                    sums = small_pool.tile([128, NQT], F32, tag="sums")
                    p_tiles = []
                    for qt in range(NQT):
                        bt = bias_pool.tile([128, N_NODE], F16, tag="bias")
                        nc.sync.dma_start(bt[:], bias_d[gh, qt * 128:(qt + 1) * 128, :])
                        sc = ps_sc.tile([128, N_NODE], F32, tag="sc")
                        nc.tensor.matmul(sc[:], q_sb[:, base + qt * 128: base + (qt + 1) * 128],
                                         k_sb[:, base: base + N_NODE], start=True, stop=False)
                        nc.tensor.matmul(sc[:], ident[:], bt[:], start=False, stop=True)
                        p = prob_pool.tile([128, N_NODE], F16, tag="p")
                        nc.scalar.activation(p[:], sc[:], AF.Exp, accum_out=sums[:, qt:qt + 1])
                        p_tiles.append(p)
                    r = small_pool.tile([128, NQT], F32, tag="r")
                    nc.vector.reciprocal(r[:], sums[:])
                    for qt in range(NQT):
                        nc.vector.tensor_scalar_mul(p_tiles[qt][:], p_tiles[qt][:], r[:, qt:qt + 1])
                    # transpose probs: pt[kt][:, qt*128:...] = p[qt][:, kt*128:...]^T
                    pt_tiles = []
                    for kt in range(NQT):
                        ptp = ps_pt.tile([128, N_NODE], F16, tag="pt")
                        for qt in range(NQT):
                            nc.tensor.transpose(ptp[:, qt * 128:(qt + 1) * 128],
                                                p_tiles[qt][:, kt * 128:(kt + 1) * 128], ident[:])
                        pt = pt_pool.tile([128, N_NODE], F16, tag="pt")
                        nc.vector.tensor_copy(pt[:], ptp[:])
                        pt_tiles.append(pt)
                    # transpose v
                    vtp = ps_vt.tile([128, NQT * HD], F16, tag="vt")
                    for kt in range(NQT):
                        nc.tensor.transpose(vtp[:, kt * HD:(kt + 1) * HD],
                                            v_sb[:, base + kt * 128: base + (kt + 1) * 128],
                                            ident[0:HD, 0:HD])
                    vt = vt_pool.tile([128, NQT * HD], F16, tag="vt")
                    nc.vector.tensor_copy(vt[:], vtp[:])
                    # attn^T = v^T @ probs^T
                    pa = ps_at.tile([HD, N_NODE], F32, tag="at")
                    for kt in range(NQT):
                        nc.tensor.matmul(pa[:], vt[:, kt * HD:(kt + 1) * HD], pt_tiles[kt][:],
                                         start=(kt == 0), stop=(kt == NQT - 1))
                    nc.scalar.activation(attn_sb[:, base: base + N_NODE], pa[:], AF.Copy)
        s_qkv.close()

        # ---------------- phase 3: out-proj + residual + mask; LN2; FFN ----------------
        wout_pool = s_wout.enter_context(tc.tile_pool(name="wout", bufs=H))
        with tc.tile_pool(name="ps_c", bufs=3, space="PSUM") as ps_c:
            wo_tiles = []
            for hh in range(H):
                wt = wout_pool.tile([HD, D], F16, tag="wout")
                nc.sync.dma_start(wt[:], wout_d[hh])
                wo_tiles.append(wt)

            # mask broadcast [128, T]
            ps_m = ps_c.tile([128, T], F32, tag="mm")
            for hf in range(2):
                nc.tensor.matmul(ps_m[:, HLF[hf]], ones[0:1, :], mask_sb[:, HLF[hf]],
                                 start=True, stop=True)
            mask_b = stat_pool.tile([128, T], F16, tag="maskb")
            nc.vector.tensor_copy(mask_b[:], ps_m[:])

            y1_tiles = []
            for m in range(KC):
                po = ps_c.tile([128, T], F32, tag="mm")
                for hf in range(2):
                    for hh in range(H):
                        nc.tensor.matmul(po[:, HLF[hf]], wo_tiles[hh][:, m * 128:(m + 1) * 128],
                                         attn_sb[:, hh * T + hf * 512: hh * T + hf * 512 + 512],
                                         start=(hh == 0), stop=(hh == H - 1))
                t = tmp_pool.tile([128, T], F16, tag="tmp")
                nc.vector.scalar_tensor_tensor(t[:], po[:], bout[:, m:m + 1], mask_b[:],
                                               op0=OP.add, op1=OP.mult)
                y1 = y1_pool.tile([128, T], F16, tag="y1")
                nc.vector.tensor_tensor(y1[:], t[:], x_tiles[m][:], op=OP.add)
                y1_tiles.append(y1)

            s_wout.close()
            s_attn.close()
            s_x.close()
            h2 = layer_norm(y1_tiles, g2, b2, ps_c, "h")

            with tc.tile_pool(name="wfc1", bufs=KC) as wfc1_pool, \
                 tc.tile_pool(name="gelu", bufs=FC) as gelu_pool:
                wf1_tiles = []
                for k in range(KC):
                    wt = wfc1_pool.tile([128, FFN], F16, tag="wfc1")
                    nc.sync.dma_start(wt[:], wfc1_d[k])
                    wf1_tiles.append(wt)
                gelu_tiles = []
                for n in range(FC):
                    pf = ps_c.tile([128, T], F32, tag="mm")
                    for hf in range(2):
                        for k in range(KC):
                            nc.tensor.matmul(pf[:, HLF[hf]], wf1_tiles[k][:, n * 128:(n + 1) * 128],
                                             h2[k][:, HLF[hf]], start=(k == 0), stop=(k == KC - 1))
                    gt = gelu_pool.tile([128, T], F16, tag="gelu")
                    nc.scalar.activation(gt[:], pf[:], AF.Gelu, bias=bfc1[:, n:n + 1])
                    gelu_tiles.append(gt)

                with tc.tile_pool(name="wfc2", bufs=FC) as wfc2_pool:
                    wf2_tiles = []
                    for kk in range(FC):
                        wt = wfc2_pool.tile([128, D], F16, tag="wfc2")
                        nc.sync.dma_start(wt[:], wfc2_d[kk])
                        wf2_tiles.append(wt)
                    for m in range(KC):
                        py = ps_c.tile([128, T], F32, tag="mm")
                        for hf in range(2):
                            for kk in range(FC):
                                nc.tensor.matmul(py[:, HLF[hf]], wf2_tiles[kk][:, m * 128:(m + 1) * 128],
                                                 gelu_tiles[kk][:, HLF[hf]],
                                                 start=(kk == 0), stop=(kk == FC - 1))
                        yo = yo_pool.tile([128, T], F32, tag="yo")
                        nc.vector.scalar_tensor_tensor(yo[:], py[:], bfc2[:, m:m + 1], y1_tiles[m][:],
                                                       op0=OP.add, op1=OP.add)
                        nc.sync.dma_start(yt_d[m], yo[:])

    nc.compile()
    return nc


def _get_runner():
    if "runner" in _cached:
        return _cached["runner"]
    import jax
    from jax.sharding import Mesh, PartitionSpec
    from jax.experimental.shard_map import shard_map
    import concourse.mybir as mybir
    from concourse.bass2jax import _bass_exec_p, install_neuronx_cc_hook, partition_id_tensor

    nc = _build()
    install_neuronx_cc_hook()
    partition_name = nc.partition_id_tensor.name if nc.partition_id_tensor else None
    in_names, out_names, out_avals, zero_outs = [], [], [], []
    for alloc in nc.m.functions[0].allocations:
        if not isinstance(alloc, mybir.MemoryLocationSet):
            continue
        name = alloc.memorylocations[0].name
        if alloc.kind == "ExternalInput":
            if name != partition_name:
                in_names.append(name)
        elif alloc.kind == "ExternalOutput":
            out_names.append(name)
            shape = tuple(alloc.tensor_shape)
            dtype = mybir.dt.np(alloc.dtype)
            out_avals.append(jax.core.ShapedArray(shape, dtype))
            zero_outs.append(np.zeros(shape, dtype))
    n_params = len(in_names)
    all_in_names = in_names + out_names + ([partition_name] if partition_name else [])

    def _body(*args):
        operands = list(args)
        if partition_name is not None:
            operands.append(partition_id_tensor())
        outs = _bass_exec_p.bind(
            *operands,
            out_avals=tuple(out_avals),
            in_names=tuple(all_in_names),
            out_names=tuple(out_names),
            lowering_input_output_aliases=(),
            sim_require_finite=False,
            sim_require_nnan=False,
            nc=nc,
        )
        return tuple(outs)

    donate = tuple(range(n_params, n_params + len(out_avals)))
    devices = jax.devices()[:NC]
    mesh = Mesh(np.asarray(devices), ("core",))
    in_specs = (PartitionSpec("core"),) * (n_params + len(out_avals))
    out_specs = (PartitionSpec("core"),) * len(out_names)
    sharded = jax.jit(
        shard_map(_body, mesh=mesh, in_specs=in_specs, out_specs=out_specs, check_rep=False),
        donate_argnums=donate, keep_unused=True,
    )

    runner = {
        "nc": nc, "sharded": sharded, "in_names": in_names,
        "out_names": out_names, "out_avals": out_avals, "zero_outs": zero_outs,
    }
    _cached["runner"] = runner
    return runner


def prep_inputs(x, attn_bias, node_non_padding_mask, in_w, in_b, out_w, out_b,
                ln1_g, ln1_b, fc1_w, fc1_b, fc2_w, fc2_b, ln2_g, ln2_b):
    """Host-side sharding/layout prep. Returns per-core dicts keyed by dram
    parameter name."""
    f16, f32 = np.float16, np.float32
    x = np.asarray(x, f32)
    xt = x.transpose(2, 1, 0).reshape(D, N_GRAPH * N_NODE).astype(f16)  # [768, 8192]
    xt_pc = [np.ascontiguousarray(xt[:, c * T:(c + 1) * T]).reshape(KC, 128, T) for c in range(NC)]
    biasb = np.asarray(attn_bias, f32).astype(f16)  # [128, 512, 512]
    mask = np.asarray(node_non_padding_mask).astype(f16)  # [16, 512]

    scale = HD ** -0.5
    in_w = np.asarray(in_w, f32).copy()
    in_b = np.asarray(in_b, f32).copy()
    in_w[:D] *= scale
    in_b[:D] *= scale
    shared = {
        "wqkv": np.ascontiguousarray(in_w.T.astype(f16)).reshape(KC, 128, 3 * D),
        "bqkv": np.ascontiguousarray(in_b.reshape(3 * H, HD).T),
        "wout": np.ascontiguousarray(np.asarray(out_w, f32).T.astype(f16)).reshape(H, HD, D),
        "bout": np.ascontiguousarray(np.asarray(out_b, f32).reshape(KC, 128).T),
        "wfc1": np.ascontiguousarray(np.asarray(fc1_w, f32).T.astype(f16)).reshape(KC, 128, FFN),
        "bfc1": np.ascontiguousarray(np.asarray(fc1_b, f32).reshape(FC, 128).T),
        "wfc2": np.ascontiguousarray(np.asarray(fc2_w, f32).T.astype(f16)).reshape(FC, 128, D),
        "bfc2": np.ascontiguousarray(np.asarray(fc2_b, f32).reshape(KC, 128).T),
        "g1": np.ascontiguousarray(np.asarray(ln1_g, f32).reshape(KC, 128).T),
        "b1": np.ascontiguousarray(np.asarray(ln1_b, f32).reshape(KC, 128).T),
        "g2": np.ascontiguousarray(np.asarray(ln2_g, f32).reshape(KC, 128).T),
        "b2": np.ascontiguousarray(np.asarray(ln2_b, f32).reshape(KC, 128).T),
        "ident": np.eye(128, dtype=f16),
        "ones": np.ones((128, 128), dtype=f16),
    }
    per_core = []
    for c in range(NC):
        m = dict(shared)
        m["xt"] = xt_pc[c]
        m["biasb"] = biasb[G * H * c: G * H * (c + 1)]
        m["maskrow"] = np.ascontiguousarray(mask[G * c: G * (c + 1)]).reshape(1, T)
        per_core.append(m)
    return per_core


def postprocess(outs):
    """outs: list of 8 per-core dicts with 'yt' [KC, 128, T] f32 -> [512, 16, 768]"""
    yt = np.stack([o["yt"].reshape(D, T) for o in outs])  # [8, 768, 1024]
    y = yt.reshape(NC, D, G, N_NODE).transpose(3, 0, 2, 1).reshape(N_NODE, N_GRAPH, D)
    return np.ascontiguousarray(y)


def run_per_core(per_core):
    r = _get_runner()
    n = NC
    concat_in = [
        np.concatenate([np.asarray(per_core[c][name]) for c in range(n)], axis=0)
        for name in r["in_names"]
    ]
    concat_zeros = [np.zeros((n * z.shape[0], *z.shape[1:]), z.dtype) for z in r["zero_outs"]]
    out_arrs = r["sharded"](*concat_in, *concat_zeros)
    return [
        {name: np.asarray(out_arrs[i]).reshape(n, *r["out_avals"][i].shape)[c]
         for i, name in enumerate(r["out_names"])}
        for c in range(n)
    ]


def kernel(**inputs):
    per_core = prep_inputs(**inputs)
    outs = run_per_core(per_core)
    return postprocess(outs)

